# revision 5
# baseline (speedup 1.0000x reference)
"""ListFoldLoss Trainium2 kernel v1 (8-core SPMD, Bass/Tile).

Same math as the baseline (rank-1 factorization of the psi matrix; see
kernel.py docstring) but phase 1 (ranks) is computed via a quantized-key
histogram instead of N brute-force compares per element:

  key_u = round(t_u * SCALE + OFF) as int16 in [0, 16384)   (14 bits)
  hi = round((key - 63.5)/128)  (= floor(key/128)),  lo = key mod 128
  r_u = #{j: key_j > key_u} = SufCntHi[hi_u] + SufG_lo[hi_u, lo_u]

Per core: one-hot the own 1024 keys' digits (is_equal vs iota), PE-matmul
them into a local 2D count histogram G_T[l, h]; AllReduce G (64KB); build
  HiCnt[h]    = sum_l G_T[l, h]                    (PE: G_T^T @ ones)
  SufRow[h]   = sum_{h'>h} HiCnt[h']               (PE: HC^T @ STRI)
  rhs2[l', h] = sum_{l>l'} G_T[l, h] + SufRow[h]   (PE: STRI^T@G_T + outer)
then per u-subtile W2 = OHlu^T @ rhs2 gives row u = SufG_lo[:, lo_u] +
SufRow[:], and a tensor_tensor_reduce pick against the hi one-hot yields
r_u exactly.  Quantization ties (elements sharing a 5.4e-4-wide key bin get
equal ranks) perturb the loss like the baseline's bf16 ties (~1e-4 rel).

Phases 1.5/2/3 are the baseline's: m = min(r, N-1-r), 32-grid step masks,
F/SufH PSUM matmuls, ReduceScatter [128,67] -> [16,67], per-core denom/ln,
host-summed partials.
"""

import numpy as np

import concourse.bacc as bacc
import concourse.bass as bass
import concourse.mybir as mybir
import concourse.tile as tile

N = 8192
NCORE = 8
P = 128
US = N // NCORE          # 1024 u's per core
UT = US // P             # 8 u-subtiles per core
NPAIR = N // 2           # 4096 loss terms
NQ = 128                 # coarse window blocks (i = 32Q + S)
NS = 32

SCALE = 1836.0
OFF = 8192.0

F32 = mybir.dt.float32
BF16 = mybir.dt.bfloat16
F16 = mybir.dt.float16
I16 = mybir.dt.int16
AF = mybir.ActivationFunctionType
OP = mybir.AluOpType


def build_module(
    debug: bool = False,
    reps: int = 1,
    collective: bool = True,
    work_bufs: int = 2,
    coll1: bool | None = None,
    coll2: bool | None = None,
):
    coll1 = collective if coll1 is None else coll1
    coll2 = collective if coll2 is None else coll2
    nc = bacc.Bacc(
        "TRN2",
        target_bir_lowering=False,
        debug=False,
        enable_asserts=False,
        num_devices=NCORE,
    )

    t_own = nc.dram_tensor("t_own", [1, US], F32, kind="ExternalInput")
    tqcol = nc.dram_tensor("tqcol", [P, UT], F32, kind="ExternalInput")
    # packed small consts: [tcol 8 | pcol 8 | win 32] per partition
    NPK = 2 * UT + NS
    packed = nc.dram_tensor("packed", [P, NPK], F32, kind="ExternalInput")
    out_part = nc.dram_tensor("out_part", [1, 1], F32, kind="ExternalOutput")
    if debug:
        dbg_r = nc.dram_tensor("dbg_r", [P, UT], F32, kind="ExternalOutput")
        dbg_m = nc.dram_tensor("dbg_m", [P, UT], F32, kind="ExternalOutput")
        dbg_g = nc.dram_tensor("dbg_g", [P, P], F16, kind="ExternalOutput")
        dbg_fh = nc.dram_tensor("dbg_fh", [P // NCORE, 67], F16, kind="ExternalOutput")

    with tile.TileContext(nc) as tc:
        with (
            tc.tile_pool(name="consts", bufs=1) as consts,
            tc.tile_pool(name="rep", bufs=2) as rp,
            tc.tile_pool(name="work", bufs=work_bufs) as work,
            tc.tile_pool(name="psA", bufs=1, space="PSUM") as psA,
            tc.tile_pool(name="psB", bufs=2, space="PSUM") as psB,
            tc.tile_pool(name="dram", bufs=2, space="DRAM") as dram,
        ):
            # ---- constant/small loads ----
            tq_sb = consts.tile([P, UT], F32)
            nc.sync.dma_start(tq_sb[:], tqcol.ap())
            packed_sb = consts.tile([P, NPK], F32)
            nc.sync.dma_start(packed_sb[:], packed.ap())
            tcol_sb = tq_sb[:]
            pcol_sb = packed_sb[:, UT : 2 * UT]
            win_sb = packed_sb[:, 2 * UT : 2 * UT + NS]

            # on-device iotas / masks (init-time only)
            iota_row_i = consts.tile([P, P], I16)
            nc.gpsimd.iota(iota_row_i[:], [[1, P]], base=0, channel_multiplier=0)
            iota_col_i = consts.tile([P, 1], I16)
            nc.gpsimd.iota(iota_col_i[:], [[0, 1]], base=0, channel_multiplier=1)
            iota_row_f = consts.tile([P, P], F32)
            nc.vector.tensor_copy(iota_row_f[:], iota_row_i[:])
            iota_col_f = consts.tile([P, 1], F32)
            nc.vector.tensor_copy(iota_col_f[:], iota_col_i[:])
            # STRI[p, f] = [p > f]  (strict lower in (p, f)); f32 for fp32 matmuls
            stri = consts.tile([P, P], F32)
            nc.vector.tensor_scalar(
                stri[:], iota_row_f[:], iota_col_f[:], None, OP.is_lt
            )

            is_i = consts.tile([P, NS], I16)
            nc.gpsimd.iota(is_i[:], [[1, NS]], base=0, channel_multiplier=0)
            iotaS = consts.tile([P, NS], F32)
            nc.vector.tensor_copy(iotaS[:], is_i[:])
            i32_i = consts.tile([P, NQ + 1], I16)
            nc.gpsimd.iota(i32_i[:], [[32, NQ + 1]], base=0, channel_multiplier=0)
            iota32e = consts.tile([P, NQ + 1], F32)
            nc.vector.tensor_copy(iota32e[:], i32_i[:])
            iotaSm = consts.tile([P, NS], F32)
            nc.vector.tensor_scalar(iotaSm[:], iotaS[:], -32.0, None, OP.add)

            stri_bf = consts.tile([P, P], BF16)
            nc.vector.tensor_copy(stri_bf[:], stri[:])
            ones_col = consts.tile([P, 1], F32)
            nc.vector.memset(ones_col[:], 1.0)
            ones_col_bf = consts.tile([P, 1], BF16)
            nc.vector.memset(ones_col_bf[:], 1.0)
            ones_row1 = consts.tile([1, P], F32)
            nc.vector.memset(ones_row1[:], 1.0)

            # own-slice weights: a = exp(p), b = exp(-p), f32 + bf16
            ab = consts.tile([P, UT, 2], F32)
            nc.scalar.activation(ab[:, :, 0], pcol_sb, AF.Exp)
            nc.scalar.activation(ab[:, :, 1], pcol_sb, AF.Exp, scale=-1.0)
            ab_bf = consts.tile([P, UT, 2], BF16)
            nc.vector.tensor_copy(ab_bf[:], ab[:])

            for _rep in range(reps):
                # ---- phase A: quantize + local histogram ----
                # broadcast own t slice (row layout) early; used for OHlu
                t_b = work.tile([P, US], F32, tag="t_b")
                nc.sync.dma_start(
                    t_b[:], t_own.ap()[0:1, :].to_broadcast((P, US))
                )
                # column-side quantization (own 1024 elements, [128, 8])
                k_col = rp.tile([P, UT], I16, tag="k_col")
                nc.vector.tensor_scalar(
                    k_col[:], tcol_sb, SCALE, OFF, OP.mult, OP.add
                )
                kf_col = rp.tile([P, UT], F32, tag="kf_col")
                nc.vector.tensor_copy(kf_col[:], k_col[:])
                hi_col_i = rp.tile([P, UT], I16, tag="hi_col_i")
                nc.vector.tensor_scalar(
                    hi_col_i[:], kf_col[:], 1.0 / 128.0, -63.5 / 128.0,
                    OP.mult, OP.add,
                )
                hi_col = rp.tile([P, UT], F32, tag="hi_col")
                nc.vector.tensor_copy(hi_col[:], hi_col_i[:])
                lo_col_i = rp.tile([P, UT], I16, tag="lo_col_i")
                nc.vector.tensor_scalar(
                    lo_col_i[:], k_col[:], 127, None, OP.bitwise_and
                )
                lo_col = rp.tile([P, UT], F32, tag="lo_col")
                nc.vector.tensor_copy(lo_col[:], lo_col_i[:])

                # j-side one-hots + G matmuls: G_T[l, h] accumulated in PSUM
                g_ps = psA.tile([P, P], F32, tag="g_ps")
                for c in range(UT):
                    ohh = work.tile([P, P], BF16, tag="ohh")
                    nc.vector.tensor_scalar(
                        ohh[:], iota_row_i[:], hi_col[:, c : c + 1], None,
                        OP.is_equal,
                    )
                    ohl = work.tile([P, P], BF16, tag="ohl")
                    nc.vector.tensor_scalar(
                        ohl[:], iota_row_i[:], lo_col[:, c : c + 1], None,
                        OP.is_equal,
                    )
                    nc.tensor.matmul(
                        g_ps[:], lhsT=ohl[:], rhs=ohh[:],
                        start=(c == 0), stop=(c == UT - 1),
                    )
                g_sb = rp.tile([P, P], F16, tag="g_sb")
                nc.vector.tensor_copy(g_sb[:], g_ps[:])

                # ---- collective 1: AllReduce G (f16: counts <= 2048 exact) ----
                g_dram = dram.tile([P, P], F16, tag="g_dram")
                nc.sync.dma_start(g_dram[:], g_sb[:])

                # row-side quantization ([128, 1024] broadcast)
                k_row = rp.tile([P, US], I16, tag="k_row")
                nc.vector.tensor_scalar(
                    k_row[:], t_b[:], SCALE, OFF, OP.mult, OP.add
                )
                lo_row = rp.tile([P, US], I16, tag="lo_row")
                nc.vector.tensor_scalar(lo_row[:], k_row[:], 127, None, OP.bitwise_and)
                ga_dram = dram.tile([P, P], F16, tag="ga_dram")
                if coll1:
                    nc.gpsimd.collective_compute(
                        "AllReduce",
                        OP.add,
                        replica_groups=[list(range(NCORE))],
                        ins=[g_dram[:].opt()],
                        outs=[ga_dram[:].opt()],
                    )
                else:  # timing-sim variant: stand-in DMA, wrong data
                    nc.sync.dma_start(ga_dram[:], g_dram[:])
                ga_sb = rp.tile([P, P], F16, tag="ga_sb")
                nc.sync.dma_start(ga_sb[:], ga_dram[:])
                if debug:
                    nc.sync.dma_start(dbg_g.ap(), ga_sb[:])

                # u-side one-hots (overlap with the collective)
                ohlu = rp.tile([P, UT, P], F32, tag="ohlu")
                ohhu = rp.tile([P, UT, P], BF16, tag="ohhu")
                for k in range(UT):
                    nc.vector.tensor_scalar(
                        ohlu[:, k, :], lo_row[:, k * P : (k + 1) * P],
                        iota_col_f[:], None, OP.is_equal,
                    )
                    nc.vector.tensor_scalar(
                        ohhu[:, k, :], iota_row_i[:], hi_col[:, k : k + 1],
                        None, OP.is_equal,
                    )

                # ---- rank tables (PE) ----
                sm_ps = psA.tile([P, 132], F32, tag="sm_ps")
                hc_ps = sm_ps[:, 0:1]
                nc.tensor.matmul(
                    hc_ps, lhsT=ga_sb[:], rhs=ones_col[:], start=True, stop=True
                )
                hc_sb = rp.tile([P, 1], F32, tag="hc_sb")
                nc.vector.tensor_copy(hc_sb[:], hc_ps)
                sufrow_ps = sm_ps[0:1, 4:132]
                nc.tensor.matmul(
                    sufrow_ps, lhsT=hc_sb[:], rhs=stri[:], start=True, stop=True
                )
                sufrow_sb = rp.tile([1, P], F32, tag="sufrow_sb")
                nc.vector.tensor_copy(sufrow_sb[:], sufrow_ps)
                rhs2_ps = psA.tile([P, P], F32, tag="rhs2_ps")
                nc.tensor.matmul(
                    rhs2_ps[:], lhsT=stri[:], rhs=ga_sb[:], start=True, stop=False
                )
                nc.tensor.matmul(
                    rhs2_ps[:], lhsT=ones_row1[:], rhs=sufrow_sb[:],
                    start=False, stop=True,
                )
                rhs2_sb = rp.tile([P, P], F32, tag="rhs2_sb")
                nc.vector.tensor_copy(rhs2_sb[:], rhs2_ps[:])

                # ---- per-element ranks: W2 matmul + hi-pick TTR ----
                rsum = rp.tile([P, UT], F32, tag="rsum")
                for k in range(UT):
                    w2_ps = psB.tile([P, P], F32, tag="w2_ps")
                    nc.tensor.matmul(
                        w2_ps[:], lhsT=ohlu[:, k, :], rhs=rhs2_sb[:],
                        start=True, stop=True,
                    )
                    scr = work.tile([P, P], F32, tag="scr_ttr")
                    nc.vector.tensor_tensor_reduce(
                        scr[:], w2_ps[:], ohhu[:, k, :], 1.0, 0.0,
                        OP.mult, OP.add, accum_out=rsum[:, k : k + 1],
                    )
                if debug:
                    nc.sync.dma_start(dbg_r.ap(), rsum[:])

                # ---- phase 1.5/2: m, q one-hots, F + Q-hist matmuls ----
                # q = floor(m/32) via round((m-15.5)/32); s = m - 32q
                # OQ[u, Q] = [q_u == Q]; F += OQ^T @ ssab;
                # Hq[Q, 2] += OQ^T @ ab;  SufH = STRI^T @ Hq (strict suffix)
                tmp = rp.tile([P, UT], F32, tag="tmp")
                m_col = rp.tile([P, UT], F32, tag="m_col")
                q_col_i = rp.tile([P, UT], I16, tag="q_col_i")
                q_col = rp.tile([P, UT], F32, tag="q_col")
                s_col = rp.tile([P, UT], F32, tag="s_col")
                f_ps = psA.tile([P, 64], F32, tag="f_ps")
                h_ps = psA.tile([P, 2], F32, tag="h_ps")

                nc.vector.tensor_scalar(
                    tmp[:], rsum[:], float(N - 1), -1.0, OP.subtract, OP.mult
                )
                nc.vector.tensor_tensor(m_col[:], rsum[:], tmp[:], OP.min)
                if debug:
                    nc.sync.dma_start(dbg_m.ap(), m_col[:])
                nc.vector.tensor_scalar(
                    q_col_i[:], m_col[:], 1.0 / 32.0, -15.5 / 32.0,
                    OP.mult, OP.add,
                )
                nc.vector.tensor_copy(q_col[:], q_col_i[:])
                nc.vector.scalar_tensor_tensor(
                    s_col[:], q_col[:], -32.0, m_col[:], OP.mult, OP.add
                )
                hq_ps = psA.tile([P, 2], F32, tag="hq_ps")
                for k in range(UT):
                    oq = work.tile([P, NQ], BF16, tag="oq")
                    nc.vector.tensor_scalar(
                        oq[:], iota_row_i[:], q_col[:, k : k + 1], None,
                        OP.is_equal,
                    )
                    ssab = work.tile([P, 2 * NS], BF16, tag="ssab")
                    nc.vector.tensor_scalar(
                        ssab[:, 0:NS], is_i[:], s_col[:, k : k + 1],
                        ab[:, k, 0:1], OP.is_le, OP.mult,
                    )
                    nc.vector.tensor_scalar(
                        ssab[:, NS : 2 * NS], is_i[:], s_col[:, k : k + 1],
                        ab[:, k, 1:2], OP.is_le, OP.mult,
                    )
                    nc.tensor.matmul(
                        f_ps[:], lhsT=oq[:], rhs=ssab[:],
                        start=(k == 0), stop=(k == UT - 1),
                    )
                    nc.tensor.matmul(
                        hq_ps[:], lhsT=oq[:], rhs=ab_bf[:, k, :],
                        start=(k == 0), stop=(k == UT - 1),
                    )
                hq_sb = rp.tile([P, 2], F32, tag="hq_sb")
                nc.vector.tensor_copy(hq_sb[:], hq_ps[:])
                nc.tensor.matmul(
                    h_ps[:], lhsT=stri[:], rhs=hq_sb[:], start=True, stop=True
                )

                # num partial: sum_u pred_u * (2*[r_u < N/2] - 1)
                sgn = rp.tile([P, UT], F32, tag="sgn")
                nc.vector.tensor_scalar(sgn[:], rsum[:], float(NPAIR), None, OP.is_lt)
                nc.vector.tensor_scalar(sgn[:], sgn[:], 2.0, -1.0, OP.mult, OP.add)
                xp = rp.tile([P, UT], F32, tag="xp")
                nc.vector.tensor_tensor(xp[:], sgn[:], pcol_sb, OP.mult)
                xq = rp.tile([P, 1], F32, tag="xq")
                nc.vector.tensor_reduce(
                    xq[:], xp[:], axis=mybir.AxisListType.X, op=OP.add
                )
                np_ps = sm_ps[0:1, 1:2]
                nc.tensor.matmul(
                    np_ps, lhsT=xq[:], rhs=ones_col[:], start=True, stop=True
                )

                fh_in = rp.tile([P, 67], F16, tag="fh_in")
                nc.vector.tensor_copy(fh_in[:, 0:64], f_ps[:])
                nc.vector.tensor_copy(fh_in[:, 64:66], h_ps[:])
                nc.vector.memset(fh_in[:, 66:67], 0.0)
                nc.vector.tensor_copy(fh_in[0:1, 66:67], np_ps)

                # ---- collective 2: ReduceScatter F/SufH/num ----
                QC = P // NCORE  # 16 Q rows per core
                fh_dram = dram.tile([P, 67], F16, tag="fh_dram")
                nc.sync.dma_start(fh_dram[:], fh_in[:])
                fhrs_dram = dram.tile([QC, 67], F16, tag="fhrs_dram")
                if coll2:
                    nc.gpsimd.collective_compute(
                        "ReduceScatter",
                        OP.add,
                        replica_groups=[list(range(NCORE))],
                        ins=[fh_dram[:].opt()],
                        outs=[fhrs_dram[:].opt()],
                    )
                else:
                    nc.sync.dma_start(fhrs_dram[:], fh_dram[0:QC, :])
                fh_sb = rp.tile([QC, 67], F16, tag="fh_sb")
                nc.sync.dma_start(fh_sb[:], fhrs_dram[:])
                if debug:
                    nc.sync.dma_start(dbg_fh.ap(), fh_sb[:])

                # ---- phase 3: denom/ln on this core's 512 windows ----
                suf3 = rp.tile([QC, 3], F32, tag="suf3")
                nc.vector.tensor_copy(suf3[:], fh_sb[:, 64:67])
                at = rp.tile([QC, NS], F32, tag="at")
                nc.vector.tensor_scalar(
                    at[:], fh_sb[:, 0:NS], suf3[:, 0:1], None, OP.add
                )
                bt = rp.tile([QC, NS], F32, tag="bt")
                nc.vector.tensor_scalar(
                    bt[:], fh_sb[:, NS : 2 * NS], suf3[:, 1:2], None, OP.add
                )
                den = rp.tile([QC, NS], F32, tag="den")
                nc.vector.tensor_tensor(den[:], at[:], bt[:], OP.mult)
                nc.vector.tensor_tensor(den[:], den[:], win_sb[0:QC, :], OP.subtract)
                # quantization ties can empty the innermost window; mirror the
                # reference's where(denom <= 0, EPS, denom) guard
                nc.vector.tensor_scalar(den[:], den[:], 1e-8, None, OP.max)
                logd = rp.tile([QC, NS], F32, tag="logd")
                lnacc = rp.tile([QC, 1], F32, tag="lnacc")
                nc.scalar.activation(logd[:], den[:], AF.Ln, accum_out=lnacc[:])
                ln_ps = sm_ps[0:1, 2:3]
                nc.tensor.matmul(
                    ln_ps, lhsT=lnacc[:], rhs=ones_col[0:QC, :],
                    start=True, stop=True,
                )
                out_sb = rp.tile([1, 1], F32, tag="out_sb")
                nc.vector.tensor_tensor(
                    out_sb[:], ln_ps, suf3[0:1, 2:3], OP.subtract
                )
                nc.sync.dma_start(out_part.ap(), out_sb[:])

    nc.compile()
    return nc


def make_in_maps(pred: np.ndarray, target: np.ndarray):
    pred = np.ascontiguousarray(pred, dtype=np.float32).reshape(N)
    target = np.ascontiguousarray(target, dtype=np.float32).reshape(N)
    in_maps = []
    for c in range(NCORE):
        tsl = target[c * US : (c + 1) * US]
        psl = pred[c * US : (c + 1) * US]
        win = np.zeros((P, NS), np.float32)
        rho = np.arange(P // NCORE)
        s_i = np.arange(NS)
        win[: P // NCORE, :] = (
            N - 64.0 * (16 * c + rho)[:, None] - 2.0 * s_i[None, :]
        )
        pk = np.concatenate(
            [tsl.reshape(UT, P).T, psl.reshape(UT, P).T, win], axis=1
        ).astype(np.float32)
        in_maps.append(
            {
                "t_own": np.ascontiguousarray(tsl.reshape(1, US)),
                "tqcol": np.ascontiguousarray(tsl.reshape(UT, P).T),
                "packed": np.ascontiguousarray(pk),
            }
        )
    return in_maps


_CACHE = {}


def _get_module():
    if "nc" not in _CACHE:
        _CACHE["nc"] = build_module(debug=False)
    return _CACHE["nc"]


def kernel(pred: np.ndarray, target: np.ndarray) -> np.ndarray:
    from concourse import bass_utils

    nc = _get_module()
    in_maps = make_in_maps(pred, target)
    res = bass_utils.run_bass_kernel_spmd(nc, in_maps, core_ids=list(range(NCORE)))
    total = np.float32(0.0)
    for c in range(NCORE):
        total = np.float32(total + res.results[c]["out_part"][0, 0])
    return np.asarray(total, dtype=np.float32)


# revision 6
# speedup vs baseline: 1.2437x; 1.2437x over previous
"""ListFoldLoss Trainium2 kernel v1 (8-core SPMD, Bass/Tile).

Same math as the baseline (rank-1 factorization of the psi matrix; see
kernel.py docstring) but phase 1 (ranks) is computed via a quantized-key
histogram instead of N brute-force compares per element:

  key_u = round(t_u * SCALE + OFF) as int16 in [0, 16384)   (14 bits)
  hi = round((key - 63.5)/128)  (= floor(key/128)),  lo = key mod 128
  r_u = #{j: key_j > key_u} = SufCntHi[hi_u] + SufG_lo[hi_u, lo_u]

Per core: one-hot the own 1024 keys' digits (is_equal vs iota), PE-matmul
them into a local 2D count histogram G_T[l, h]; AllReduce G (64KB); build
  HiCnt[h]    = sum_l G_T[l, h]                    (PE: G_T^T @ ones)
  SufRow[h]   = sum_{h'>h} HiCnt[h']               (PE: HC^T @ STRI)
  rhs2[l', h] = sum_{l>l'} G_T[l, h] + SufRow[h]   (PE: STRI^T@G_T + outer)
then per u-subtile W2 = OHlu^T @ rhs2 gives row u = SufG_lo[:, lo_u] +
SufRow[:], and a tensor_tensor_reduce pick against the hi one-hot yields
r_u exactly.  Quantization ties (elements sharing a 5.4e-4-wide key bin get
equal ranks) perturb the loss like the baseline's bf16 ties (~1e-4 rel).

Phases 1.5/2/3 are the baseline's: m = min(r, N-1-r), 32-grid step masks,
F/SufH PSUM matmuls, ReduceScatter [128,67] -> [16,67], per-core denom/ln,
host-summed partials.
"""

import numpy as np

import concourse.bacc as bacc
import concourse.bass as bass
import concourse.mybir as mybir
import concourse.tile as tile

N = 8192
NCORE = 8
P = 128
US = N // NCORE          # 1024 u's per core
UT = US // P             # 8 u-subtiles per core
NPAIR = N // 2           # 4096 loss terms
NQ = 128                 # coarse window blocks (i = 32Q + S)
NS = 32

SCALE = 918.0          # 13-bit keys: 64 hi-bins x 128 lo-bins
OFF = 4096.0
NH = 64

F32 = mybir.dt.float32
BF16 = mybir.dt.bfloat16
F16 = mybir.dt.float16
I16 = mybir.dt.int16
AF = mybir.ActivationFunctionType
OP = mybir.AluOpType


def build_module(
    debug: bool = False,
    reps: int = 1,
    collective: bool = True,
    work_bufs: int = 2,
    coll1: bool | None = None,
    coll2: bool | None = None,
):
    coll1 = collective if coll1 is None else coll1
    coll2 = collective if coll2 is None else coll2
    nc = bacc.Bacc(
        "TRN2",
        target_bir_lowering=False,
        debug=False,
        enable_asserts=False,
        num_devices=NCORE,
    )

    t_own = nc.dram_tensor("t_own", [1, US], F32, kind="ExternalInput")
    tqcol = nc.dram_tensor("tqcol", [P, UT], F32, kind="ExternalInput")
    # packed small consts: [tcol 8 | pcol 8 | win 32] per partition
    NPK = 2 * UT + NS
    packed = nc.dram_tensor("packed", [P, NPK], F32, kind="ExternalInput")
    out_part = nc.dram_tensor("out_part", [1, 1], F32, kind="ExternalOutput")
    if debug:
        dbg_r = nc.dram_tensor("dbg_r", [P, UT], F32, kind="ExternalOutput")
        dbg_m = nc.dram_tensor("dbg_m", [P, UT], F32, kind="ExternalOutput")
        dbg_g = nc.dram_tensor("dbg_g", [P, NH], F16, kind="ExternalOutput")
        dbg_fh = nc.dram_tensor("dbg_fh", [P // NCORE, 67], F16, kind="ExternalOutput")

    with tile.TileContext(nc) as tc:
        with (
            tc.tile_pool(name="consts", bufs=1) as consts,
            tc.tile_pool(name="rep", bufs=2) as rp,
            tc.tile_pool(name="work", bufs=work_bufs) as work,
            tc.tile_pool(name="psA", bufs=1, space="PSUM") as psA,
            tc.tile_pool(name="psB", bufs=2, space="PSUM") as psB,
            tc.tile_pool(name="dram", bufs=2, space="DRAM") as dram,
        ):
            # ---- constant/small loads ----
            tq_sb = consts.tile([P, UT], F32)
            nc.sync.dma_start(tq_sb[:], tqcol.ap())
            packed_sb = consts.tile([P, NPK], F32)
            nc.sync.dma_start(packed_sb[:], packed.ap())
            tcol_sb = tq_sb[:]
            pcol_sb = packed_sb[:, UT : 2 * UT]
            win_sb = packed_sb[:, 2 * UT : 2 * UT + NS]

            # on-device iotas / masks (init-time only)
            iota_row_i = consts.tile([P, P], I16)
            nc.gpsimd.iota(iota_row_i[:], [[1, P]], base=0, channel_multiplier=0)
            iota_col_i = consts.tile([P, 1], I16)
            nc.gpsimd.iota(iota_col_i[:], [[0, 1]], base=0, channel_multiplier=1)
            iota_row_f = consts.tile([P, P], F32)
            nc.vector.tensor_copy(iota_row_f[:], iota_row_i[:])
            iota_col_f = consts.tile([P, 1], F32)
            nc.vector.tensor_copy(iota_col_f[:], iota_col_i[:])
            # STRI[p, f] = [p > f]  (strict lower in (p, f)); f32 for fp32 matmuls
            stri = consts.tile([P, P], F32)
            nc.vector.tensor_scalar(
                stri[:], iota_row_f[:], iota_col_f[:], None, OP.is_lt
            )

            is_i = consts.tile([P, NS], I16)
            nc.gpsimd.iota(is_i[:], [[1, NS]], base=0, channel_multiplier=0)
            iotaS = consts.tile([P, NS], F32)
            nc.vector.tensor_copy(iotaS[:], is_i[:])
            i32_i = consts.tile([P, NQ + 1], I16)
            nc.gpsimd.iota(i32_i[:], [[32, NQ + 1]], base=0, channel_multiplier=0)
            iota32e = consts.tile([P, NQ + 1], F32)
            nc.vector.tensor_copy(iota32e[:], i32_i[:])
            iotaSm = consts.tile([P, NS], F32)
            nc.vector.tensor_scalar(iotaSm[:], iotaS[:], -32.0, None, OP.add)

            stri_bf = consts.tile([P, P], BF16)
            nc.vector.tensor_copy(stri_bf[:], stri[:])
            ones_col = consts.tile([P, 1], F32)
            nc.vector.memset(ones_col[:], 1.0)
            ones_col_bf = consts.tile([P, 1], BF16)
            nc.vector.memset(ones_col_bf[:], 1.0)
            ones_row1 = consts.tile([1, P], F32)
            nc.vector.memset(ones_row1[:], 1.0)

            # own-slice weights: a = exp(p), b = exp(-p), f32 + bf16
            ab = consts.tile([P, UT, 2], F32)
            nc.scalar.activation(ab[:, :, 0], pcol_sb, AF.Exp)
            nc.scalar.activation(ab[:, :, 1], pcol_sb, AF.Exp, scale=-1.0)
            ab_bf = consts.tile([P, UT, 2], BF16)
            nc.vector.tensor_copy(ab_bf[:], ab[:])

            for _rep in range(reps):
                # ---- phase A: quantize + local histogram ----
                # broadcast own t slice (row layout) early; used for OHlu
                t_b = work.tile([P, US], F32, tag="t_b")
                nc.sync.dma_start(
                    t_b[:], t_own.ap()[0:1, :].to_broadcast((P, US))
                )
                # column-side quantization (own 1024 elements, [128, 8])
                k_col = rp.tile([P, UT], I16, tag="k_col")
                nc.vector.tensor_scalar(
                    k_col[:], tcol_sb, SCALE, OFF, OP.mult, OP.add
                )
                kf_col = rp.tile([P, UT], F32, tag="kf_col")
                nc.vector.tensor_copy(kf_col[:], k_col[:])
                hi_col_i = rp.tile([P, UT], I16, tag="hi_col_i")
                nc.vector.tensor_scalar(
                    hi_col_i[:], kf_col[:], 1.0 / 128.0, -63.5 / 128.0,
                    OP.mult, OP.add,
                )
                hi_col = rp.tile([P, UT], F32, tag="hi_col")
                nc.vector.tensor_copy(hi_col[:], hi_col_i[:])
                lo_col_i = rp.tile([P, UT], I16, tag="lo_col_i")
                nc.vector.tensor_scalar(
                    lo_col_i[:], k_col[:], 127, None, OP.bitwise_and
                )
                lo_col = rp.tile([P, UT], F32, tag="lo_col")
                nc.vector.tensor_copy(lo_col[:], lo_col_i[:])

                # j-side one-hots + G matmuls: G_T[l, h] accumulated in PSUM
                g_ps = psA.tile([P, NH], F32, tag="g_ps")
                for c in range(UT):
                    ohh = work.tile([P, NH], BF16, tag="ohh")
                    nc.vector.tensor_scalar(
                        ohh[:], iota_row_i[:, 0:NH], hi_col[:, c : c + 1],
                        None, OP.is_equal,
                    )
                    ohl = work.tile([P, P], BF16, tag="ohl")
                    nc.vector.tensor_scalar(
                        ohl[:], iota_row_i[:], lo_col[:, c : c + 1], None,
                        OP.is_equal,
                    )
                    nc.tensor.matmul(
                        g_ps[:], lhsT=ohl[:], rhs=ohh[:],
                        start=(c == 0), stop=(c == UT - 1),
                    )
                g_sb = rp.tile([P, NH], F16, tag="g_sb")
                nc.vector.tensor_copy(g_sb[:], g_ps[:])

                # ---- collective 1: AllReduce G (f16: counts <= 2048 exact) ----
                g_dram = dram.tile([P, NH], F16, tag="g_dram")
                nc.sync.dma_start(g_dram[:], g_sb[:])

                # row-side quantization ([128, 1024] broadcast)
                k_row = rp.tile([P, US], I16, tag="k_row")
                nc.vector.tensor_scalar(
                    k_row[:], t_b[:], SCALE, OFF, OP.mult, OP.add
                )
                lo_row = rp.tile([P, US], I16, tag="lo_row")
                nc.vector.tensor_scalar(lo_row[:], k_row[:], 127, None, OP.bitwise_and)
                ga_dram = dram.tile([P, NH], F16, tag="ga_dram")
                if coll1:
                    nc.gpsimd.collective_compute(
                        "AllReduce",
                        OP.add,
                        replica_groups=[list(range(NCORE))],
                        ins=[g_dram[:].opt()],
                        outs=[ga_dram[:].opt()],
                    )
                else:  # timing-sim variant: stand-in DMA, wrong data
                    nc.sync.dma_start(ga_dram[:], g_dram[:])
                ga_sb = rp.tile([P, NH], F16, tag="ga_sb")
                nc.sync.dma_start(ga_sb[:], ga_dram[:])
                if debug:
                    nc.sync.dma_start(dbg_g.ap(), ga_sb[:])

                # u-side one-hots (overlap with the collective)
                ohlu = rp.tile([P, UT, P], F32, tag="ohlu")
                ohhu = rp.tile([P, UT, P], BF16, tag="ohhu")
                for k in range(UT):
                    nc.vector.tensor_scalar(
                        ohlu[:, k, :], lo_row[:, k * P : (k + 1) * P],
                        iota_col_f[:], None, OP.is_equal,
                    )
                    nc.vector.tensor_scalar(
                        ohhu[:, k, :], iota_row_i[:], hi_col[:, k : k + 1],
                        None, OP.is_equal,
                    )

                # ---- rank tables (PE) ----
                sm_ps = psA.tile([P, 132], F32, tag="sm_ps")
                hc_ps = sm_ps[:, 0:1]
                nc.tensor.matmul(
                    hc_ps, lhsT=ga_sb[:], rhs=ones_col[:], start=True, stop=True
                )
                hc_sb = rp.tile([P, 1], F32, tag="hc_sb")
                nc.vector.tensor_copy(hc_sb[:], hc_ps)
                sufrow_ps = sm_ps[0:1, 4:132]
                nc.tensor.matmul(
                    sufrow_ps, lhsT=hc_sb[:], rhs=stri[:], start=True, stop=True
                )
                sufrow_sb = rp.tile([1, P], F32, tag="sufrow_sb")
                nc.vector.tensor_copy(sufrow_sb[:], sufrow_ps)
                rhs2_ps = psA.tile([P, P], F32, tag="rhs2_ps")
                nc.tensor.matmul(
                    rhs2_ps[:], lhsT=stri[:], rhs=ga_sb[:], start=True, stop=False
                )
                nc.tensor.matmul(
                    rhs2_ps[:], lhsT=ones_row1[:], rhs=sufrow_sb[:],
                    start=False, stop=True,
                )
                rhs2_sb = rp.tile([P, P], F32, tag="rhs2_sb")
                nc.vector.tensor_copy(rhs2_sb[:], rhs2_ps[:])

                # ---- per-element ranks: W2 matmul + hi-pick TTR ----
                rsum = rp.tile([P, UT], F32, tag="rsum")
                for k in range(UT):
                    w2_ps = psB.tile([P, P], F32, tag="w2_ps")
                    nc.tensor.matmul(
                        w2_ps[:], lhsT=ohlu[:, k, :], rhs=rhs2_sb[:],
                        start=True, stop=True,
                    )
                    scr = work.tile([P, P], F32, tag="scr_ttr")
                    nc.vector.tensor_tensor_reduce(
                        scr[:], w2_ps[:], ohhu[:, k, :], 1.0, 0.0,
                        OP.mult, OP.add, accum_out=rsum[:, k : k + 1],
                    )
                if debug:
                    nc.sync.dma_start(dbg_r.ap(), rsum[:])

                # ---- phase 1.5/2: m, q one-hots, F + Q-hist matmuls ----
                # q = floor(m/32) via round((m-15.5)/32); s = m - 32q
                # OQ[u, Q] = [q_u == Q]; F += OQ^T @ ssab;
                # Hq[Q, 2] += OQ^T @ ab;  SufH = STRI^T @ Hq (strict suffix)
                tmp = rp.tile([P, UT], F32, tag="tmp")
                m_col = rp.tile([P, UT], F32, tag="m_col")
                q_col_i = rp.tile([P, UT], I16, tag="q_col_i")
                q_col = rp.tile([P, UT], F32, tag="q_col")
                s_col = rp.tile([P, UT], F32, tag="s_col")
                f_ps = psA.tile([P, 64], F32, tag="f_ps")
                h_ps = psA.tile([P, 2], F32, tag="h_ps")

                nc.vector.tensor_scalar(
                    tmp[:], rsum[:], float(N - 1), -1.0, OP.subtract, OP.mult
                )
                nc.vector.tensor_tensor(m_col[:], rsum[:], tmp[:], OP.min)
                if debug:
                    nc.sync.dma_start(dbg_m.ap(), m_col[:])
                nc.vector.tensor_scalar(
                    q_col_i[:], m_col[:], 1.0 / 32.0, -15.5 / 32.0,
                    OP.mult, OP.add,
                )
                nc.vector.tensor_copy(q_col[:], q_col_i[:])
                nc.vector.scalar_tensor_tensor(
                    s_col[:], q_col[:], -32.0, m_col[:], OP.mult, OP.add
                )
                hq_ps = psA.tile([P, 2], F32, tag="hq_ps")
                for k in range(UT):
                    oq = work.tile([P, NQ], BF16, tag="oq")
                    nc.vector.tensor_scalar(
                        oq[:], iota_row_i[:], q_col[:, k : k + 1], None,
                        OP.is_equal,
                    )
                    ssab = work.tile([P, 2 * NS], BF16, tag="ssab")
                    nc.vector.tensor_scalar(
                        ssab[:, 0:NS], is_i[:], s_col[:, k : k + 1],
                        ab[:, k, 0:1], OP.is_le, OP.mult,
                    )
                    nc.vector.tensor_scalar(
                        ssab[:, NS : 2 * NS], is_i[:], s_col[:, k : k + 1],
                        ab[:, k, 1:2], OP.is_le, OP.mult,
                    )
                    nc.tensor.matmul(
                        f_ps[:], lhsT=oq[:], rhs=ssab[:],
                        start=(k == 0), stop=(k == UT - 1),
                    )
                    nc.tensor.matmul(
                        hq_ps[:], lhsT=oq[:], rhs=ab_bf[:, k, :],
                        start=(k == 0), stop=(k == UT - 1),
                    )
                hq_sb = rp.tile([P, 2], F32, tag="hq_sb")
                nc.vector.tensor_copy(hq_sb[:], hq_ps[:])
                nc.tensor.matmul(
                    h_ps[:], lhsT=stri[:], rhs=hq_sb[:], start=True, stop=True
                )

                # num partial: sum_u pred_u * (2*[r_u < N/2] - 1)
                sgn = rp.tile([P, UT], F32, tag="sgn")
                nc.vector.tensor_scalar(sgn[:], rsum[:], float(NPAIR), None, OP.is_lt)
                nc.vector.tensor_scalar(sgn[:], sgn[:], 2.0, -1.0, OP.mult, OP.add)
                xp = rp.tile([P, UT], F32, tag="xp")
                nc.vector.tensor_tensor(xp[:], sgn[:], pcol_sb, OP.mult)
                xq = rp.tile([P, 1], F32, tag="xq")
                nc.vector.tensor_reduce(
                    xq[:], xp[:], axis=mybir.AxisListType.X, op=OP.add
                )
                np_ps = sm_ps[0:1, 1:2]
                nc.tensor.matmul(
                    np_ps, lhsT=xq[:], rhs=ones_col[:], start=True, stop=True
                )

                fh_in = rp.tile([P, 67], F16, tag="fh_in")
                nc.vector.tensor_copy(fh_in[:, 0:64], f_ps[:])
                nc.vector.tensor_copy(fh_in[:, 64:66], h_ps[:])
                nc.vector.memset(fh_in[:, 66:67], 0.0)
                nc.vector.tensor_copy(fh_in[0:1, 66:67], np_ps)

                # ---- collective 2: ReduceScatter F/SufH/num ----
                QC = P // NCORE  # 16 Q rows per core
                fh_dram = dram.tile([P, 67], F16, tag="fh_dram")
                nc.sync.dma_start(fh_dram[:], fh_in[:])
                fhrs_dram = dram.tile([QC, 67], F16, tag="fhrs_dram")
                if coll2:
                    nc.gpsimd.collective_compute(
                        "ReduceScatter",
                        OP.add,
                        replica_groups=[list(range(NCORE))],
                        ins=[fh_dram[:].opt()],
                        outs=[fhrs_dram[:].opt()],
                    )
                else:
                    nc.sync.dma_start(fhrs_dram[:], fh_dram[0:QC, :])
                fh_sb = rp.tile([QC, 67], F16, tag="fh_sb")
                nc.sync.dma_start(fh_sb[:], fhrs_dram[:])
                if debug:
                    nc.sync.dma_start(dbg_fh.ap(), fh_sb[:])

                # ---- phase 3: denom/ln on this core's 512 windows ----
                suf3 = rp.tile([QC, 3], F32, tag="suf3")
                nc.vector.tensor_copy(suf3[:], fh_sb[:, 64:67])
                at = rp.tile([QC, NS], F32, tag="at")
                nc.vector.tensor_scalar(
                    at[:], fh_sb[:, 0:NS], suf3[:, 0:1], None, OP.add
                )
                bt = rp.tile([QC, NS], F32, tag="bt")
                nc.vector.tensor_scalar(
                    bt[:], fh_sb[:, NS : 2 * NS], suf3[:, 1:2], None, OP.add
                )
                den = rp.tile([QC, NS], F32, tag="den")
                nc.vector.tensor_tensor(den[:], at[:], bt[:], OP.mult)
                nc.vector.tensor_tensor(den[:], den[:], win_sb[0:QC, :], OP.subtract)
                # quantization ties can empty the innermost window; mirror the
                # reference's where(denom <= 0, EPS, denom) guard
                nc.vector.tensor_scalar(den[:], den[:], 1e-8, None, OP.max)
                logd = rp.tile([QC, NS], F32, tag="logd")
                lnacc = rp.tile([QC, 1], F32, tag="lnacc")
                nc.scalar.activation(logd[:], den[:], AF.Ln, accum_out=lnacc[:])
                ln_ps = sm_ps[0:1, 2:3]
                nc.tensor.matmul(
                    ln_ps, lhsT=lnacc[:], rhs=ones_col[0:QC, :],
                    start=True, stop=True,
                )
                out_sb = rp.tile([1, 1], F32, tag="out_sb")
                nc.vector.tensor_tensor(
                    out_sb[:], ln_ps, suf3[0:1, 2:3], OP.subtract
                )
                nc.sync.dma_start(out_part.ap(), out_sb[:])

    nc.compile()
    return nc


def make_in_maps(pred: np.ndarray, target: np.ndarray):
    pred = np.ascontiguousarray(pred, dtype=np.float32).reshape(N)
    target = np.ascontiguousarray(target, dtype=np.float32).reshape(N)
    in_maps = []
    for c in range(NCORE):
        tsl = target[c * US : (c + 1) * US]
        psl = pred[c * US : (c + 1) * US]
        win = np.zeros((P, NS), np.float32)
        rho = np.arange(P // NCORE)
        s_i = np.arange(NS)
        win[: P // NCORE, :] = (
            N - 64.0 * (16 * c + rho)[:, None] - 2.0 * s_i[None, :]
        )
        pk = np.concatenate(
            [tsl.reshape(UT, P).T, psl.reshape(UT, P).T, win], axis=1
        ).astype(np.float32)
        in_maps.append(
            {
                "t_own": np.ascontiguousarray(tsl.reshape(1, US)),
                "tqcol": np.ascontiguousarray(tsl.reshape(UT, P).T),
                "packed": np.ascontiguousarray(pk),
            }
        )
    return in_maps


_CACHE = {}


def _get_module():
    if "nc" not in _CACHE:
        _CACHE["nc"] = build_module(debug=False)
    return _CACHE["nc"]


def kernel(pred: np.ndarray, target: np.ndarray) -> np.ndarray:
    from concourse import bass_utils

    nc = _get_module()
    in_maps = make_in_maps(pred, target)
    res = bass_utils.run_bass_kernel_spmd(nc, in_maps, core_ids=list(range(NCORE)))
    total = np.float32(0.0)
    for c in range(NCORE):
        total = np.float32(total + res.results[c]["out_part"][0, 0])
    return np.asarray(total, dtype=np.float32)


# revision 7
# speedup vs baseline: 1.2693x; 1.0206x over previous
"""ListFoldLoss Trainium2 kernel v1 (8-core SPMD, Bass/Tile).

Same math as the baseline (rank-1 factorization of the psi matrix; see
kernel.py docstring) but phase 1 (ranks) is computed via a quantized-key
histogram instead of N brute-force compares per element:

  key_u = round(t_u * SCALE + OFF) as int16 in [0, 16384)   (14 bits)
  hi = round((key - 63.5)/128)  (= floor(key/128)),  lo = key mod 128
  r_u = #{j: key_j > key_u} = SufCntHi[hi_u] + SufG_lo[hi_u, lo_u]

Per core: one-hot the own 1024 keys' digits (is_equal vs iota), PE-matmul
them into a local 2D count histogram G_T[l, h]; AllReduce G (64KB); build
  HiCnt[h]    = sum_l G_T[l, h]                    (PE: G_T^T @ ones)
  SufRow[h]   = sum_{h'>h} HiCnt[h']               (PE: HC^T @ STRI)
  rhs2[l', h] = sum_{l>l'} G_T[l, h] + SufRow[h]   (PE: STRI^T@G_T + outer)
then per u-subtile W2 = OHlu^T @ rhs2 gives row u = SufG_lo[:, lo_u] +
SufRow[:], and a tensor_tensor_reduce pick against the hi one-hot yields
r_u exactly.  Quantization ties (elements sharing a 5.4e-4-wide key bin get
equal ranks) perturb the loss like the baseline's bf16 ties (~1e-4 rel).

Phases 1.5/2/3 are the baseline's: m = min(r, N-1-r), 32-grid step masks,
F/SufH PSUM matmuls, ReduceScatter [128,67] -> [16,67], per-core denom/ln,
host-summed partials.
"""

import numpy as np

import concourse.bacc as bacc
import concourse.bass as bass
import concourse.mybir as mybir
import concourse.tile as tile

N = 8192
NCORE = 8
P = 128
US = N // NCORE          # 1024 u's per core
UT = US // P             # 8 u-subtiles per core
NPAIR = N // 2           # 4096 loss terms
NQ = 128                 # coarse window blocks (i = 32Q + S)
NS = 32

SCALE = 918.0          # 13-bit keys: 64 hi-bins x 128 lo-bins
OFF = 4096.0
NH = 64

F32 = mybir.dt.float32
BF16 = mybir.dt.bfloat16
F16 = mybir.dt.float16
I16 = mybir.dt.int16
AF = mybir.ActivationFunctionType
OP = mybir.AluOpType


def build_module(
    debug: bool = False,
    reps: int = 1,
    collective: bool = True,
    work_bufs: int = 2,
    coll1: bool | None = None,
    coll2: bool | None = None,
):
    coll1 = collective if coll1 is None else coll1
    coll2 = collective if coll2 is None else coll2
    nc = bacc.Bacc(
        "TRN2",
        target_bir_lowering=False,
        debug=False,
        enable_asserts=False,
        num_devices=NCORE,
    )

    t_own = nc.dram_tensor("t_own", [1, US], F32, kind="ExternalInput")
    tqcol = nc.dram_tensor("tqcol", [P, UT], F32, kind="ExternalInput")
    # packed small consts: [tcol 8 | pcol 8 | win 32] per partition
    NPK = 2 * UT + NS
    packed = nc.dram_tensor("packed", [P, NPK], F32, kind="ExternalInput")
    out_part = nc.dram_tensor("out_part", [1, 1], F32, kind="ExternalOutput")
    if debug:
        dbg_r = nc.dram_tensor("dbg_r", [P, UT], F32, kind="ExternalOutput")
        dbg_m = nc.dram_tensor("dbg_m", [P, UT], F32, kind="ExternalOutput")
        dbg_g = nc.dram_tensor("dbg_g", [P, NH], F16, kind="ExternalOutput")
        dbg_fh = nc.dram_tensor("dbg_fh", [P // NCORE, 67], F16, kind="ExternalOutput")

    with tile.TileContext(nc) as tc:
        with (
            tc.tile_pool(name="consts", bufs=1) as consts,
            tc.tile_pool(name="rep", bufs=2) as rp,
            tc.tile_pool(name="work", bufs=work_bufs) as work,
            tc.tile_pool(name="psA", bufs=1, space="PSUM") as psA,
            tc.tile_pool(name="psB", bufs=2, space="PSUM") as psB,
            tc.tile_pool(name="dram", bufs=2, space="DRAM") as dram,
        ):
            # ---- constant/small loads ----
            tq_sb = consts.tile([P, UT], F32)
            nc.sync.dma_start(tq_sb[:], tqcol.ap())
            packed_sb = consts.tile([P, NPK], F32)
            nc.sync.dma_start(packed_sb[:], packed.ap())
            tcol_sb = tq_sb[:]
            pcol_sb = packed_sb[:, UT : 2 * UT]
            win_sb = packed_sb[:, 2 * UT : 2 * UT + NS]

            # on-device iotas / masks (init-time only)
            iota_row_i = consts.tile([P, P], I16)
            nc.gpsimd.iota(iota_row_i[:], [[1, P]], base=0, channel_multiplier=0)
            iota_col_i = consts.tile([P, 1], I16)
            nc.gpsimd.iota(iota_col_i[:], [[0, 1]], base=0, channel_multiplier=1)
            iota_row_f = consts.tile([P, P], F32)
            nc.vector.tensor_copy(iota_row_f[:], iota_row_i[:])
            iota_col_f = consts.tile([P, 1], F32)
            nc.vector.tensor_copy(iota_col_f[:], iota_col_i[:])
            # STRI[p, f] = [p > f]  (strict lower in (p, f)); f32 for fp32 matmuls
            stri = consts.tile([P, P], F32)
            nc.vector.tensor_scalar(
                stri[:], iota_row_f[:], iota_col_f[:], None, OP.is_lt
            )

            is_i = consts.tile([P, NS], I16)
            nc.gpsimd.iota(is_i[:], [[1, NS]], base=0, channel_multiplier=0)

            stri_bf = consts.tile([P, P], BF16)
            nc.vector.tensor_copy(stri_bf[:], stri[:])
            ones_col = consts.tile([P, 1], F32)
            nc.vector.memset(ones_col[:], 1.0)
            ones_col_bf = consts.tile([P, 1], BF16)
            nc.vector.memset(ones_col_bf[:], 1.0)
            ones_row1 = consts.tile([1, P], F32)
            nc.vector.memset(ones_row1[:], 1.0)

            # own-slice weights: a = exp(p), b = exp(-p), f32 + bf16
            ab = consts.tile([P, UT, 2], F32)
            nc.scalar.activation(ab[:, :, 0], pcol_sb, AF.Exp)
            nc.scalar.activation(ab[:, :, 1], pcol_sb, AF.Exp, scale=-1.0)
            ab_bf = consts.tile([P, UT, 2], BF16)
            nc.vector.tensor_copy(ab_bf[:], ab[:])

            for _rep in range(reps):
                # ---- phase A: quantize + local histogram ----
                # broadcast own t slice (row layout) early; used for OHlu
                t_b = work.tile([P, US], F32, tag="t_b")
                nc.sync.dma_start(
                    t_b[:], t_own.ap()[0:1, :].to_broadcast((P, US))
                )
                # column-side quantization (own 1024 elements, [128, 8])
                k_col = rp.tile([P, UT], I16, tag="k_col")
                nc.vector.tensor_scalar(
                    k_col[:], tcol_sb, SCALE, OFF, OP.mult, OP.add
                )
                kf_col = rp.tile([P, UT], F32, tag="kf_col")
                nc.vector.tensor_copy(kf_col[:], k_col[:])
                hi_col_i = rp.tile([P, UT], I16, tag="hi_col_i")
                nc.vector.tensor_scalar(
                    hi_col_i[:], kf_col[:], 1.0 / 128.0, -63.5 / 128.0,
                    OP.mult, OP.add,
                )
                hi_col = rp.tile([P, UT], F32, tag="hi_col")
                nc.vector.tensor_copy(hi_col[:], hi_col_i[:])
                lo_col_i = rp.tile([P, UT], I16, tag="lo_col_i")
                nc.vector.tensor_scalar(
                    lo_col_i[:], k_col[:], 127, None, OP.bitwise_and
                )
                lo_col = rp.tile([P, UT], F32, tag="lo_col")
                nc.vector.tensor_copy(lo_col[:], lo_col_i[:])

                # j-side one-hots + G matmuls: G_T[l, h] accumulated in PSUM
                g_ps = psA.tile([P, NH], F32, tag="g_ps")
                for c in range(UT):
                    ohh = work.tile([P, NH], BF16, tag="ohh")
                    nc.vector.tensor_scalar(
                        ohh[:], iota_row_i[:, 0:NH], hi_col[:, c : c + 1],
                        None, OP.is_equal,
                    )
                    ohl = work.tile([P, P], BF16, tag="ohl")
                    nc.vector.tensor_scalar(
                        ohl[:], iota_row_i[:], lo_col[:, c : c + 1], None,
                        OP.is_equal,
                    )
                    nc.tensor.matmul(
                        g_ps[:], lhsT=ohl[:], rhs=ohh[:],
                        start=(c == 0), stop=(c == UT - 1),
                    )
                g_sb = rp.tile([P, NH], F16, tag="g_sb")
                nc.vector.tensor_copy(g_sb[:], g_ps[:])

                # ---- collective 1: AllReduce G (f16: counts <= 2048 exact) ----
                g_dram = dram.tile([P, NH], F16, tag="g_dram")
                nc.sync.dma_start(g_dram[:], g_sb[:])

                # row-side quantization ([128, 1024] broadcast)
                k_row = rp.tile([P, US], I16, tag="k_row")
                nc.vector.tensor_scalar(
                    k_row[:], t_b[:], SCALE, OFF, OP.mult, OP.add
                )
                lo_row = rp.tile([P, US], I16, tag="lo_row")
                nc.vector.tensor_scalar(lo_row[:], k_row[:], 127, None, OP.bitwise_and)
                ga_dram = dram.tile([P, NH], F16, tag="ga_dram")
                if coll1:
                    nc.gpsimd.collective_compute(
                        "AllReduce",
                        OP.add,
                        replica_groups=[list(range(NCORE))],
                        ins=[g_dram[:].opt()],
                        outs=[ga_dram[:].opt()],
                    )
                else:  # timing-sim variant: stand-in DMA, wrong data
                    nc.sync.dma_start(ga_dram[:], g_dram[:])
                ga_sb = rp.tile([P, NH], F16, tag="ga_sb")
                nc.sync.dma_start(ga_sb[:], ga_dram[:])
                if debug:
                    nc.sync.dma_start(dbg_g.ap(), ga_sb[:])

                # u-side one-hots (overlap with the collective)
                ohlu = rp.tile([P, UT, P], F32, tag="ohlu")
                ohhu = rp.tile([P, UT, P], BF16, tag="ohhu")
                for k in range(UT):
                    nc.vector.tensor_scalar(
                        ohlu[:, k, :], lo_row[:, k * P : (k + 1) * P],
                        iota_col_f[:], None, OP.is_equal,
                    )
                    nc.vector.tensor_scalar(
                        ohhu[:, k, :], iota_row_i[:], hi_col[:, k : k + 1],
                        None, OP.is_equal,
                    )

                # ---- rank tables (PE) ----
                sm_ps = psA.tile([P, 132], F32, tag="sm_ps")
                hc_ps = sm_ps[:, 0:1]
                nc.tensor.matmul(
                    hc_ps, lhsT=ga_sb[:], rhs=ones_col[:], start=True, stop=True
                )
                hc_sb = rp.tile([P, 1], F32, tag="hc_sb")
                nc.vector.tensor_copy(hc_sb[:], hc_ps)
                sufrow_ps = sm_ps[0:1, 4:132]
                nc.tensor.matmul(
                    sufrow_ps, lhsT=hc_sb[:], rhs=stri[:], start=True, stop=True
                )
                sufrow_sb = rp.tile([1, P], F32, tag="sufrow_sb")
                nc.vector.tensor_copy(sufrow_sb[:], sufrow_ps)
                rhs2_ps = psA.tile([P, P], F32, tag="rhs2_ps")
                nc.tensor.matmul(
                    rhs2_ps[:], lhsT=stri[:], rhs=ga_sb[:], start=True, stop=False
                )
                nc.tensor.matmul(
                    rhs2_ps[:], lhsT=ones_row1[:], rhs=sufrow_sb[:],
                    start=False, stop=True,
                )
                rhs2_sb = rp.tile([P, P], F32, tag="rhs2_sb")
                nc.vector.tensor_copy(rhs2_sb[:], rhs2_ps[:])

                # ---- per-element ranks: W2 matmul + hi-pick TTR ----
                rsum = rp.tile([P, UT], F32, tag="rsum")
                for k in range(UT):
                    w2_ps = psB.tile([P, P], F32, tag="w2_ps")
                    nc.tensor.matmul(
                        w2_ps[:], lhsT=ohlu[:, k, :], rhs=rhs2_sb[:],
                        start=True, stop=True,
                    )
                    scr = work.tile([P, P], F32, tag="scr_ttr")
                    nc.vector.tensor_tensor_reduce(
                        scr[:], w2_ps[:], ohhu[:, k, :], 1.0, 0.0,
                        OP.mult, OP.add, accum_out=rsum[:, k : k + 1],
                    )
                if debug:
                    nc.sync.dma_start(dbg_r.ap(), rsum[:])

                # ---- phase 1.5/2: m, q one-hots, F + Q-hist matmuls ----
                # q = floor(m/32) via round((m-15.5)/32); s = m - 32q
                # OQ[u, Q] = [q_u == Q]; F += OQ^T @ ssab;
                # Hq[Q, 2] += OQ^T @ ab;  SufH = STRI^T @ Hq (strict suffix)
                tmp = rp.tile([P, UT], F32, tag="tmp")
                m_col = rp.tile([P, UT], F32, tag="m_col")
                q_col_i = rp.tile([P, UT], I16, tag="q_col_i")
                q_col = rp.tile([P, UT], F32, tag="q_col")
                s_col = rp.tile([P, UT], F32, tag="s_col")
                f_ps = psA.tile([P, 64], F32, tag="f_ps")
                h_ps = psA.tile([P, 2], F32, tag="h_ps")

                nc.vector.tensor_scalar(
                    tmp[:], rsum[:], float(N - 1), -1.0, OP.subtract, OP.mult
                )
                nc.vector.tensor_tensor(m_col[:], rsum[:], tmp[:], OP.min)
                if debug:
                    nc.sync.dma_start(dbg_m.ap(), m_col[:])
                nc.vector.tensor_scalar(
                    q_col_i[:], m_col[:], 1.0 / 32.0, -15.5 / 32.0,
                    OP.mult, OP.add,
                )
                nc.vector.tensor_copy(q_col[:], q_col_i[:])
                nc.vector.scalar_tensor_tensor(
                    s_col[:], q_col[:], -32.0, m_col[:], OP.mult, OP.add
                )
                hq_ps = psA.tile([P, 2], F32, tag="hq_ps")
                for k in range(UT):
                    oq = work.tile([P, NQ], BF16, tag="oq")
                    nc.vector.tensor_scalar(
                        oq[:], iota_row_i[:], q_col[:, k : k + 1], None,
                        OP.is_equal,
                    )
                    ssab = work.tile([P, 2 * NS], BF16, tag="ssab")
                    nc.vector.tensor_scalar(
                        ssab[:, 0:NS], is_i[:], s_col[:, k : k + 1],
                        ab[:, k, 0:1], OP.is_le, OP.mult,
                    )
                    nc.vector.tensor_scalar(
                        ssab[:, NS : 2 * NS], is_i[:], s_col[:, k : k + 1],
                        ab[:, k, 1:2], OP.is_le, OP.mult,
                    )
                    nc.tensor.matmul(
                        f_ps[:], lhsT=oq[:], rhs=ssab[:],
                        start=(k == 0), stop=(k == UT - 1),
                    )
                    nc.tensor.matmul(
                        hq_ps[:], lhsT=oq[:], rhs=ab_bf[:, k, :],
                        start=(k == 0), stop=(k == UT - 1),
                    )
                hq_sb = rp.tile([P, 2], F32, tag="hq_sb")
                nc.vector.tensor_copy(hq_sb[:], hq_ps[:])
                nc.tensor.matmul(
                    h_ps[:], lhsT=stri[:], rhs=hq_sb[:], start=True, stop=True
                )

                # num partial: sum_u pred_u * (2*[r_u < N/2] - 1)
                sgn = rp.tile([P, UT], F32, tag="sgn")
                nc.vector.tensor_scalar(sgn[:], rsum[:], float(NPAIR), None, OP.is_lt)
                nc.vector.tensor_scalar(sgn[:], sgn[:], 2.0, -1.0, OP.mult, OP.add)
                xp = rp.tile([P, UT], F32, tag="xp")
                nc.vector.tensor_tensor(xp[:], sgn[:], pcol_sb, OP.mult)
                xq = rp.tile([P, 1], F32, tag="xq")
                nc.vector.tensor_reduce(
                    xq[:], xp[:], axis=mybir.AxisListType.X, op=OP.add
                )
                np_ps = sm_ps[0:1, 1:2]
                nc.tensor.matmul(
                    np_ps, lhsT=xq[:], rhs=ones_col[:], start=True, stop=True
                )

                fh_in = rp.tile([P, 67], F16, tag="fh_in")
                nc.vector.tensor_copy(fh_in[:, 0:64], f_ps[:])
                nc.vector.tensor_copy(fh_in[:, 64:66], h_ps[:])
                nc.vector.memset(fh_in[:, 66:67], 0.0)
                nc.vector.tensor_copy(fh_in[0:1, 66:67], np_ps)

                # ---- collective 2: ReduceScatter F/SufH/num ----
                QC = P // NCORE  # 16 Q rows per core
                fh_dram = dram.tile([P, 67], F16, tag="fh_dram")
                nc.sync.dma_start(fh_dram[:], fh_in[:])
                fhrs_dram = dram.tile([QC, 67], F16, tag="fhrs_dram")
                if coll2:
                    nc.gpsimd.collective_compute(
                        "ReduceScatter",
                        OP.add,
                        replica_groups=[list(range(NCORE))],
                        ins=[fh_dram[:].opt()],
                        outs=[fhrs_dram[:].opt()],
                    )
                else:
                    nc.sync.dma_start(fhrs_dram[:], fh_dram[0:QC, :])
                fh_sb = rp.tile([QC, 67], F16, tag="fh_sb")
                nc.sync.dma_start(fh_sb[:], fhrs_dram[:])
                if debug:
                    nc.sync.dma_start(dbg_fh.ap(), fh_sb[:])

                # ---- phase 3: denom/ln on this core's 512 windows ----
                suf3 = rp.tile([QC, 3], F32, tag="suf3")
                nc.vector.tensor_copy(suf3[:], fh_sb[:, 64:67])
                bt = rp.tile([QC, NS], F32, tag="bt")
                nc.vector.tensor_scalar(
                    bt[:], fh_sb[:, NS : 2 * NS], suf3[:, 1:2], None, OP.add
                )
                den = rp.tile([QC, NS], F32, tag="den")
                nc.vector.scalar_tensor_tensor(
                    den[:], fh_sb[:, 0:NS], suf3[:, 0:1], bt[:],
                    OP.add, OP.mult,
                )
                nc.vector.tensor_tensor(den[:], den[:], win_sb[0:QC, :], OP.subtract)
                # quantization ties can empty the innermost window; mirror the
                # reference's where(denom <= 0, EPS, denom) guard
                nc.vector.tensor_scalar(den[:], den[:], 1e-8, None, OP.max)
                logd = rp.tile([QC, NS], F32, tag="logd")
                lnacc = rp.tile([QC, 1], F32, tag="lnacc")
                nc.scalar.activation(logd[:], den[:], AF.Ln, accum_out=lnacc[:])
                ln_ps = sm_ps[0:1, 2:3]
                nc.tensor.matmul(
                    ln_ps, lhsT=lnacc[:], rhs=ones_col[0:QC, :],
                    start=True, stop=True,
                )
                out_sb = rp.tile([1, 1], F32, tag="out_sb")
                nc.vector.tensor_tensor(
                    out_sb[:], ln_ps, suf3[0:1, 2:3], OP.subtract
                )
                nc.sync.dma_start(out_part.ap(), out_sb[:])

    nc.compile()
    return nc


def make_in_maps(pred: np.ndarray, target: np.ndarray):
    pred = np.ascontiguousarray(pred, dtype=np.float32).reshape(N)
    target = np.ascontiguousarray(target, dtype=np.float32).reshape(N)
    in_maps = []
    for c in range(NCORE):
        tsl = target[c * US : (c + 1) * US]
        psl = pred[c * US : (c + 1) * US]
        win = np.zeros((P, NS), np.float32)
        rho = np.arange(P // NCORE)
        s_i = np.arange(NS)
        win[: P // NCORE, :] = (
            N - 64.0 * (16 * c + rho)[:, None] - 2.0 * s_i[None, :]
        )
        pk = np.concatenate(
            [tsl.reshape(UT, P).T, psl.reshape(UT, P).T, win], axis=1
        ).astype(np.float32)
        in_maps.append(
            {
                "t_own": np.ascontiguousarray(tsl.reshape(1, US)),
                "tqcol": np.ascontiguousarray(tsl.reshape(UT, P).T),
                "packed": np.ascontiguousarray(pk),
            }
        )
    return in_maps


_CACHE = {}


def _get_module():
    if "nc" not in _CACHE:
        _CACHE["nc"] = build_module(debug=False)
    return _CACHE["nc"]


def kernel(pred: np.ndarray, target: np.ndarray) -> np.ndarray:
    from concourse import bass_utils

    nc = _get_module()
    in_maps = make_in_maps(pred, target)
    res = bass_utils.run_bass_kernel_spmd(nc, in_maps, core_ids=list(range(NCORE)))
    total = np.float32(0.0)
    for c in range(NCORE):
        total = np.float32(total + res.results[c]["out_part"][0, 0])
    return np.asarray(total, dtype=np.float32)
